# revision 41
# baseline (speedup 1.0000x reference)
"""AGNNConv distributed Bass kernel for 8 TRN2 NeuronCores (v2: matmul reduce).

out = (1+eps)*feat + h,  h[d] = sum_{e: dst_e=d} p_e * norm_feat[src_e]
with p_e = edge-softmax grouped by src.

Algebra (softmax max-subtraction dropped -- identity in exact math):
    z_n = sum_{e: src_e=n} exp(beta*ew_e)
    g_n = feat_n / (||feat_n|| * z_n)            # per-node row scale
    h_d = sum_{e: dst_e=d} exp(beta*ew_e) * g[src_e]
    out = (1+eps)*feat + h

Core c owns dst nodes [c*6250, (c+1)*6250). Inputs replicated via in_maps
(free); every core builds the full g table (bf16) locally, then per dst-tile
(128 dst nodes, edges host-grouped + padded):
  1) dma_gather g rows for the tile's edges (bf16, 2 table halves for int16)
  2) one-hot via TensorE bit-trick: M[e,n] = #matching bits of (dst_e, n)
     over 7 bit-planes (K=14 matmul); S = relu(w*M - 6w) fused on ScalarE
     (per-edge w enters via ACT scale/bias) -> w-weighted one-hot, bf16
  3) h_tile (PSUM f32) += S_t^T @ msg_t accumulated over edge tiles
  4) out_tile = h + (1+eps)*feat_my_tile, DMA straight out
No scatter DMA at all: Q7 descriptor generation only for the gather.
"""

import sys

sys.path.insert(0, "/opt/trn_rl_repo")

import numpy as np

N, E, D = 50000, 640000, 128
NCORES = 8
SH = N // NCORES            # 6250 dst nodes per core
NT = (N + 127) // 128       # 391 node tiles (g table)
HALFP = 64 * NT             # permuted-table half base (rows with p<64)
NPAD = NT * 128             # 50048
HTILES = (SH + 127) // 128  # 49 dst tiles per core

PAD_EW = -80.0              # exp(beta*PAD_EW) == 0 (inside ACT LUT range)


def _host_prep(src, dst, edge_weight):
    """Index/layout prep only (no float math on tensor values)."""
    src = np.asarray(src).astype(np.int64)
    dst = np.asarray(dst).astype(np.int64)
    ew = np.asarray(edge_weight).astype(np.float32)

    # ---- global z padding (per-node src-grouped edge weights, fixed K) ----
    deg = np.bincount(src, minlength=NPAD)
    K = int(deg.max())
    order = np.argsort(src, kind="stable")
    starts = np.zeros(NPAD + 1, np.int64)
    np.cumsum(deg, out=starts[1:])
    slot = np.arange(E, dtype=np.int64) - starts[src[order]]
    zpad = np.full((NPAD, K), PAD_EW, np.float32)
    zpad[src[order], slot] = ew[order]
    zpad_dev = np.ascontiguousarray(
        zpad.reshape(NT, 128, K).transpose(1, 0, 2).reshape(128, NT * K)
    )

    # ---- per-core edge grouping by (dst tile, src half) ----
    owner = dst // SH
    dstl = dst - owner * SH           # 0..SH-1
    dtile = dstl // 128               # dst tile
    dbit = dstl % 128                 # within-tile dst row
    srcp = (src % 128) * NT + src // 128  # permuted g-table row
    halfsel = (src % 128 >= 64).astype(np.int64)  # 0 -> rows < HALFP

    counts = np.zeros((NCORES, HTILES, 2), np.int64)
    np.add.at(counts, (owner, dtile, halfsel), 1)
    net = (counts.max(axis=0) + 127) // 128  # [HTILES, 2] edge tiles
    net = np.maximum(net, 1)
    tet = int(net.sum())

    # segment order: (half, tile)
    netT = net.T.reshape(-1)  # [2*HTILES] in (half, tile) order
    seg_off = np.zeros(HTILES * 2, np.int64)
    seg_off[1:] = np.cumsum(netT)[:-1]

    EPAD = tet * 128

    core_arrays = []
    for c in range(NCORES):
        m = np.nonzero(owner == c)[0]
        key = halfsel[m] * HTILES + dtile[m]
        korder = np.argsort(key, kind="stable")
        me = m[korder]
        keys = key[korder]
        kb = np.r_[0, np.nonzero(np.diff(keys))[0] + 1]
        sf = np.zeros(len(keys), np.int64)
        sf[kb] = kb
        np.maximum.accumulate(sf, out=sf)
        within = np.arange(len(keys)) - sf
        pos = seg_off[keys] * 128 + within

        gidx = np.zeros(EPAD, np.int16)          # pads gather row 0 (valid)
        ewp = np.full(EPAD, PAD_EW, np.float32)  # pads weight -> 0
        bits = np.zeros(EPAD, np.int64)

        gbase = halfsel[me] * HALFP
        gidx[pos] = (srcp[me] - gbase).astype(np.int16)
        ewp[pos] = ew[me]
        bits[pos] = dbit[me]

        g16 = gidx.reshape(tet * 8, 16).T        # wrap-16: [16, tet*8]
        g128 = np.tile(g16, (8, 1))              # replicated [128, tet*8]
        ewd = ewp.reshape(tet, 128).T            # [128, tet]

        bp = np.zeros((14, EPAD), np.float32)
        for b in range(7):
            bb = (bits >> b) & 1
            bp[2 * b + 1] = bb
            bp[2 * b] = 1 - bb
        core_arrays.append(
            dict(
                gidx=np.ascontiguousarray(g128),
                ewp=np.ascontiguousarray(ewd),
                bitp=np.ascontiguousarray(bp),
            )
        )

    import ml_dtypes

    nb = np.zeros((14, 128), np.float32)
    nn = np.arange(128)
    for b in range(7):
        bb = (nn >> b) & 1
        nb[2 * b + 1] = bb
        nb[2 * b] = 1 - bb
    nb = nb.astype(ml_dtypes.bfloat16)
    for ca in core_arrays:
        ca["bitp"] = ca["bitp"].astype(ml_dtypes.bfloat16)

    maxc = counts.max(axis=0)  # [HTILES, 2]
    nidx_exact = np.minimum((maxc + 15) // 16 * 16, net * 128)
    nidx_exact = np.maximum(nidx_exact, 16)

    return zpad_dev, core_arrays, nb, net, nidx_exact, K


def _perm_table(feat_pad):
    return np.ascontiguousarray(
        feat_pad.reshape(NT, 128, D).transpose(1, 0, 2).reshape(128, NT * D)
    )


_COMPILED = {}


def _build(net, nidx_exact, K):
    import concourse.bass as bass
    import concourse.bacc as bacc
    from concourse import mybir, tile

    f32 = mybir.dt.float32
    bf16 = mybir.dt.bfloat16
    i16 = mybir.dt.int16
    AF = mybir.ActivationFunctionType
    ALU = mybir.AluOpType

    tet = int(net.sum())
    nmax = int(net.sum(axis=1).max())

    nc = bacc.Bacc(None, debug=False, num_swdge_queues=1)

    feat_ext = nc.dram_tensor("feat", [128, NT * D], f32, kind="ExternalInput")
    featmy_ext = nc.dram_tensor("feat_my", [SH, D], f32, kind="ExternalInput")
    zpad_ext = nc.dram_tensor("zpad", [128, NT * K], f32, kind="ExternalInput")
    gidx_ext = nc.dram_tensor("gidx", [128, tet * 8], i16, kind="ExternalInput")
    ewp_ext = nc.dram_tensor("ewp", [128, tet], f32, kind="ExternalInput")
    bitp_ext = nc.dram_tensor("bitp", [14, tet * 128], bf16, kind="ExternalInput")
    nbits_ext = nc.dram_tensor("nbits", [14, 128], bf16, kind="ExternalInput")
    beta_ext = nc.dram_tensor("beta", [1, 1], f32, kind="ExternalInput")
    eps_ext = nc.dram_tensor("eps", [1, 1], f32, kind="ExternalInput")
    out_ext = nc.dram_tensor("out", [SH, D], f32, kind="ExternalOutput")

    g_dram = nc.dram_tensor("g_table", [128, NT * D], bf16)

    with tile.TileContext(nc) as tc:
        with (
            tc.tile_pool(name="persist", bufs=1) as pp,
            tc.tile_pool(name="fst", bufs=3) as fpool,
            tc.tile_pool(name="gst", bufs=3) as gpool,
            tc.tile_pool(name="sq", bufs=3) as sqpool,
            tc.tile_pool(name="msg", bufs=4) as mpool,
            tc.tile_pool(name="bitpool", bufs=8) as bpool,
            tc.tile_pool(name="stair", bufs=8) as spool,
            tc.tile_pool(name="outp", bufs=3) as opool,
            tc.tile_pool(name="mpsum", bufs=6, space="PSUM") as mpsum,
            tc.tile_pool(name="hpsum", bufs=2, space="PSUM") as hpsum,
        ):
            # ---------- scalars ----------
            beta_s = pp.tile([1, 1], f32, tag="beta_s")
            eps_s = pp.tile([1, 1], f32, tag="eps_s")
            nc.sync.dma_start(out=beta_s[:], in_=beta_ext[:])
            nc.sync.dma_start(out=eps_s[:], in_=eps_ext[:])
            beta_b = pp.tile([128, 1], f32, tag="beta_b")
            ep1_b = pp.tile([128, 1], f32, tag="ep1_b")
            nc.gpsimd.partition_broadcast(beta_b[:], beta_s[:])
            nc.gpsimd.partition_broadcast(ep1_b[:], eps_s[:])
            nc.vector.tensor_scalar_add(ep1_b[:], ep1_b[:], 1.0)

            # ---------- z ----------
            ztile = pp.tile([128, NT, K], f32, tag="ztile")
            nc.sync.dma_start(
                out=ztile[:].rearrange("p a b -> p (a b)"), in_=zpad_ext[:]
            )
            nc.scalar.activation(
                ztile[:].rearrange("p a b -> p (a b)"),
                ztile[:].rearrange("p a b -> p (a b)"),
                AF.Exp,
                scale=beta_b[:],
            )
            zvec = pp.tile([128, NT], f32, tag="zvec")
            nc.vector.tensor_reduce(zvec[:], ztile[:], mybir.AxisListType.X, ALU.add)
            zrec = pp.tile([128, NT], f32, tag="zrec")
            nc.vector.tensor_scalar_max(zvec[:], zvec[:], 1e-30)
            nc.vector.reciprocal(zrec[:], zvec[:])

            # ---------- g table (bf16), half 0 (partitions 0:64) first ----
            svec = pp.tile([128, NT], f32, tag="svec")
            st_widths = []
            t0 = 0
            while t0 < NT:
                w = min(16, NT - t0)
                st_widths.append((t0, w))
                t0 += w
            for p0, p1 in ((0, 64), (64, 128)):
                for t0, w in st_widths:
                    ft = fpool.tile([128, 16, D], f32, tag="ft")
                    nc.sync.dma_start(
                        out=ft[p0:p1, :w, :],
                        in_=feat_ext[p0:p1, t0 * D : (t0 + w) * D],
                    )
                    sq = sqpool.tile([128, 16, D], f32, tag="sq")
                    nc.scalar.activation(
                        sq[p0:p1, :w, :].rearrange("p a b -> p (a b)"),
                        ft[p0:p1, :w, :].rearrange("p a b -> p (a b)"),
                        AF.Square,
                    )
                    ss = svec[p0:p1, t0 : t0 + w]
                    nc.vector.tensor_reduce(
                        ss, sq[p0:p1, :w, :], mybir.AxisListType.X, ALU.add
                    )
                    nc.vector.tensor_scalar_max(ss, ss, 1e-30)
                    nc.scalar.activation(ss, ss, AF.Sqrt)
                    nc.vector.reciprocal(ss, ss)
                    nc.vector.tensor_tensor(
                        ss, ss, zrec[p0:p1, t0 : t0 + w], ALU.mult
                    )
                    gt = gpool.tile([128, 16, D], bf16, tag="gt")
                    for j in range(w):
                        if j % 2 == 0:
                            nc.vector.tensor_scalar_mul(
                                gt[p0:p1, j, :],
                                ft[p0:p1, j, :],
                                svec[p0:p1, t0 + j : t0 + j + 1],
                            )
                        else:
                            nc.scalar.activation(
                                gt[p0:p1, j, :],
                                ft[p0:p1, j, :],
                                AF.Copy,
                                scale=svec[p0:p1, t0 + j : t0 + j + 1],
                            )
                    nc.sync.dma_start(
                        out=g_dram[p0:p1, t0 * D : (t0 + w) * D],
                        in_=gt[p0:p1, :w, :],
                    )

            # ---------- per-edge weights + bit planes ----------
            wvec = pp.tile([128, tet], f32, tag="wvec")
            nc.sync.dma_start(out=wvec[:], in_=ewp_ext[:])
            nc.scalar.activation(wvec[:], wvec[:], AF.Exp, scale=beta_b[:])
            neg6w = pp.tile([128, tet], f32, tag="neg6w")
            nc.vector.tensor_scalar_mul(neg6w[:], wvec[:], -6.0)

            nbits = pp.tile([14, 128], bf16, tag="nbits")
            nc.sync.dma_start(out=nbits[:], in_=nbits_ext[:])

            gidx_t = pp.tile([128, tet * 8], i16, tag="gidx_t")
            nc.sync.dma_start(out=gidx_t[:], in_=gidx_ext[:])

            g_rows = g_dram[:].rearrange("p (c f) -> (p c) f", f=D)

            # ---------- edge phase: two sweeps (one per table half) -----
            h0sb = pp.tile([128, HTILES, D], f32, tag="h0sb")
            nmax0 = int(net[:, 0].max())
            nmax1 = int(net[:, 1].max())
            # zero all msg slots once: rows skipped by exact-count gathers
            # must stay finite (0 * 0 = 0 in the S@msg matmul)

            T = 0
            gc = 0
            for hh, nmaxh in ((0, nmax0), (1, nmax1)):
                base = 0 if hh == 0 else HALFP
                for i in range(HTILES):
                    nh = int(net[i, hh])
                    msg = mpool.tile([128, max(nmax0, nmax1), D], bf16, tag="msg")
                    nidx = int(nidx_exact[i, hh])
                    nhq = (nidx + 127) // 128
                    if nidx % 128:
                        # rows the exact-count gather skips must be finite;
                        # zero the last column, the gather overwrites the
                        # valid prefix rows
                        nc.vector.memset(msg[:, nhq - 1, :], 0.0)
                    nc.gpsimd.dma_gather(
                        msg[:, :nhq, :],
                        g_rows[base : base + HALFP, :],
                        gidx_t[:, gc : gc + (nidx + 15) // 16],
                        nidx,
                        nidx,
                        D,
                        queue_num=0,
                    )
                    gc += nh * 8
                    hp = hpsum.tile([128, D], f32, tag="hp")
                    bp = bpool.tile([14, max(nmax0, nmax1), 128], bf16, tag="bp")
                    nc.sync.dma_start(
                        out=bp[:, :nh, :],
                        in_=bitp_ext[:, T * 128 : (T + nh) * 128],
                    )
                    for t in range(nh):
                        mp = mpsum.tile([128, 128], f32, tag="mp")
                        nc.tensor.matmul(
                            mp[:], bp[:, t, :], nbits[:], start=True, stop=True
                        )
                        st = spool.tile([128, 128], bf16, tag="st")
                        nc.scalar.activation(
                            st[:],
                            mp[:],
                            AF.Relu,
                            bias=neg6w[:, T + t : T + t + 1],
                            scale=wvec[:, T + t : T + t + 1],
                        )
                        nc.tensor.matmul(
                            hp[:],
                            st[:],
                            msg[:, t, :],
                            start=(t == 0),
                            stop=(t == nh - 1),
                        )
                    T += nh
                    if hh == 0:
                        nc.vector.tensor_copy(h0sb[:, i, :], hp[:])
                    else:
                        rows = min(128, SH - i * 128)
                        ftm = opool.tile([128, D], f32, tag="ftm")
                        ot = opool.tile([128, D], f32, tag="ot")
                        nc.sync.dma_start(
                            out=ftm[:rows, :],
                            in_=featmy_ext[i * 128 : i * 128 + rows, :],
                        )
                        nc.vector.tensor_scalar_mul(
                            ftm[:rows, :], ftm[:rows, :], ep1_b[:rows, :]
                        )
                        nc.vector.tensor_tensor(
                            ot[:rows, :], ftm[:rows, :], hp[:rows, :], ALU.add
                        )
                        nc.vector.tensor_tensor(
                            ot[:rows, :], ot[:rows, :], h0sb[:rows, i, :], ALU.add
                        )
                        nc.sync.dma_start(
                            out=out_ext[i * 128 : i * 128 + rows, :],
                            in_=ot[:rows, :],
                        )

    nc.finalize()
    return nc


def kernel(feat, edge_weight, src, dst, beta, eps):
    from concourse.bass_utils import run_bass_kernel_spmd

    feat = np.asarray(feat, dtype=np.float32)
    ew = np.asarray(edge_weight, dtype=np.float32)
    beta = np.asarray(beta, dtype=np.float32)
    eps = np.asarray(eps, dtype=np.float32)

    zpad_dev, core_arrays, nb, net, nidx_exact, K = _host_prep(src, dst, ew)

    key = (
        K,
        tuple(int(x) for x in net.reshape(-1)),
        tuple(int(x) for x in nidx_exact.reshape(-1)),
    )
    if key not in _COMPILED:
        _COMPILED[key] = _build(net, nidx_exact, K)
    nc = _COMPILED[key]

    feat_pad = np.zeros((NPAD, D), np.float32)
    feat_pad[:N] = feat
    feat_perm = _perm_table(feat_pad)
    beta2 = beta.reshape(1, 1)
    eps2 = eps.reshape(1, 1)

    in_maps = []
    for c in range(NCORES):
        ca = core_arrays[c]
        in_maps.append(
            {
                "feat": feat_perm,
                "feat_my": np.ascontiguousarray(feat[c * SH : (c + 1) * SH]),
                "zpad": zpad_dev,
                "gidx": ca["gidx"],
                "ewp": ca["ewp"],
                "bitp": ca["bitp"],
                "nbits": nb,
                "beta": beta2,
                "eps": eps2,
            }
        )

    res = run_bass_kernel_spmd(nc, in_maps, core_ids=list(range(NCORES)))
    out = np.concatenate([res.results[c]["out"] for c in range(NCORES)], axis=0)
    return out.astype(np.float32)


# revision 42
# speedup vs baseline: 1.0271x; 1.0271x over previous
"""AGNNConv distributed Bass kernel for 8 TRN2 NeuronCores (v2: matmul reduce).

out = (1+eps)*feat + h,  h[d] = sum_{e: dst_e=d} p_e * norm_feat[src_e]
with p_e = edge-softmax grouped by src.

Algebra (softmax max-subtraction dropped -- identity in exact math):
    z_n = sum_{e: src_e=n} exp(beta*ew_e)
    g_n = feat_n / (||feat_n|| * z_n)            # per-node row scale
    h_d = sum_{e: dst_e=d} exp(beta*ew_e) * g[src_e]
    out = (1+eps)*feat + h

Core c owns dst nodes [c*6250, (c+1)*6250). Inputs replicated via in_maps
(free); every core builds the full g table (bf16) locally, then per dst-tile
(128 dst nodes, edges host-grouped + padded):
  1) dma_gather g rows for the tile's edges (bf16, 2 table halves for int16)
  2) one-hot via TensorE bit-trick: M[e,n] = #matching bits of (dst_e, n)
     over 7 bit-planes (K=14 matmul); S = relu(w*M - 6w) fused on ScalarE
     (per-edge w enters via ACT scale/bias) -> w-weighted one-hot, bf16
  3) h_tile (PSUM f32) += S_t^T @ msg_t accumulated over edge tiles
  4) out_tile = h + (1+eps)*feat_my_tile, DMA straight out
No scatter DMA at all: Q7 descriptor generation only for the gather.
"""

import sys

sys.path.insert(0, "/opt/trn_rl_repo")

import numpy as np

N, E, D = 50000, 640000, 128
NCORES = 8
SH = N // NCORES            # 6250 dst nodes per core
NT = (N + 127) // 128       # 391 node tiles (g table)
HALFP = 64 * NT             # permuted-table half base (rows with p<64)
NPAD = NT * 128             # 50048
HTILES = (SH + 127) // 128  # 49 dst tiles per core

PAD_EW = -80.0              # exp(beta*PAD_EW) == 0 (inside ACT LUT range)


def _host_prep(src, dst, edge_weight):
    """Index/layout prep only (no float math on tensor values)."""
    src = np.asarray(src).astype(np.int64)
    dst = np.asarray(dst).astype(np.int64)
    ew = np.asarray(edge_weight).astype(np.float32)

    # ---- global z padding (per-node src-grouped edge weights, fixed K) ----
    deg = np.bincount(src, minlength=NPAD)
    K = int(deg.max())
    order = np.argsort(src, kind="stable")
    starts = np.zeros(NPAD + 1, np.int64)
    np.cumsum(deg, out=starts[1:])
    slot = np.arange(E, dtype=np.int64) - starts[src[order]]
    zpad = np.full((NPAD, K), PAD_EW, np.float32)
    zpad[src[order], slot] = ew[order]
    zpad_dev = np.ascontiguousarray(
        zpad.reshape(NT, 128, K).transpose(1, 0, 2).reshape(128, NT * K)
    )

    # ---- per-core edge grouping by (dst tile, src half) ----
    owner = dst // SH
    dstl = dst - owner * SH           # 0..SH-1
    dtile = dstl // 128               # dst tile
    dbit = dstl % 128                 # within-tile dst row
    srcp = (src % 128) * NT + src // 128  # permuted g-table row
    halfsel = (src % 128 >= 64).astype(np.int64)  # 0 -> rows < HALFP

    counts = np.zeros((NCORES, HTILES, 2), np.int64)
    np.add.at(counts, (owner, dtile, halfsel), 1)
    net = (counts.max(axis=0) + 127) // 128  # [HTILES, 2] edge tiles
    net = np.maximum(net, 1)
    tet = int(net.sum())

    # segment order: (half, tile)
    netT = net.T.reshape(-1)  # [2*HTILES] in (half, tile) order
    seg_off = np.zeros(HTILES * 2, np.int64)
    seg_off[1:] = np.cumsum(netT)[:-1]

    EPAD = tet * 128

    core_arrays = []
    for c in range(NCORES):
        m = np.nonzero(owner == c)[0]
        key = halfsel[m] * HTILES + dtile[m]
        korder = np.argsort(key, kind="stable")
        me = m[korder]
        keys = key[korder]
        kb = np.r_[0, np.nonzero(np.diff(keys))[0] + 1]
        sf = np.zeros(len(keys), np.int64)
        sf[kb] = kb
        np.maximum.accumulate(sf, out=sf)
        within = np.arange(len(keys)) - sf
        pos = seg_off[keys] * 128 + within

        gidx = np.zeros(EPAD, np.int16)          # pads gather row 0 (valid)
        ewp = np.full(EPAD, PAD_EW, np.float32)  # pads weight -> 0
        bits = np.zeros(EPAD, np.int64)

        gbase = halfsel[me] * HALFP
        gidx[pos] = (srcp[me] - gbase).astype(np.int16)
        ewp[pos] = ew[me]
        bits[pos] = dbit[me]

        g16 = gidx.reshape(tet * 8, 16).T        # wrap-16: [16, tet*8]
        g128 = np.tile(g16, (8, 1))              # replicated [128, tet*8]
        ewd = ewp.reshape(tet, 128).T            # [128, tet]

        bp = np.zeros((14, EPAD), np.float32)
        for b in range(7):
            bb = (bits >> b) & 1
            bp[2 * b + 1] = bb
            bp[2 * b] = 1 - bb
        core_arrays.append(
            dict(
                gidx=np.ascontiguousarray(g128),
                ewp=np.ascontiguousarray(ewd),
                bitp=np.ascontiguousarray(bp),
            )
        )

    import ml_dtypes

    nb = np.zeros((14, 128), np.float32)
    nn = np.arange(128)
    for b in range(7):
        bb = (nn >> b) & 1
        nb[2 * b + 1] = bb
        nb[2 * b] = 1 - bb
    nb = nb.astype(ml_dtypes.bfloat16)
    for ca in core_arrays:
        ca["bitp"] = ca["bitp"].astype(ml_dtypes.bfloat16)

    maxc = counts.max(axis=0)  # [HTILES, 2]
    nidx_exact = np.minimum((maxc + 15) // 16 * 16, net * 128)
    nidx_exact = np.maximum(nidx_exact, 16)

    return zpad_dev, core_arrays, nb, net, nidx_exact, K


def _perm_table(feat_pad):
    return np.ascontiguousarray(
        feat_pad.reshape(NT, 128, D).transpose(1, 0, 2).reshape(128, NT * D)
    )


_COMPILED = {}


def _build(net, nidx_exact, K):
    import concourse.bass as bass
    import concourse.bacc as bacc
    from concourse import mybir, tile

    f32 = mybir.dt.float32
    bf16 = mybir.dt.bfloat16
    i16 = mybir.dt.int16
    AF = mybir.ActivationFunctionType
    ALU = mybir.AluOpType

    tet = int(net.sum())
    nmax = int(net.sum(axis=1).max())

    nc = bacc.Bacc(None, debug=False, num_swdge_queues=1)

    feat_ext = nc.dram_tensor("feat", [128, NT * D], f32, kind="ExternalInput")
    featmy_ext = nc.dram_tensor("feat_my", [SH, D], f32, kind="ExternalInput")
    zpad_ext = nc.dram_tensor("zpad", [128, NT * K], f32, kind="ExternalInput")
    gidx_ext = nc.dram_tensor("gidx", [128, tet * 8], i16, kind="ExternalInput")
    ewp_ext = nc.dram_tensor("ewp", [128, tet], f32, kind="ExternalInput")
    bitp_ext = nc.dram_tensor("bitp", [14, tet * 128], bf16, kind="ExternalInput")
    nbits_ext = nc.dram_tensor("nbits", [14, 128], bf16, kind="ExternalInput")
    beta_ext = nc.dram_tensor("beta", [1, 1], f32, kind="ExternalInput")
    eps_ext = nc.dram_tensor("eps", [1, 1], f32, kind="ExternalInput")
    out_ext = nc.dram_tensor("out", [SH, D], f32, kind="ExternalOutput")

    g_dram = nc.dram_tensor("g_table", [128, NT * D], bf16)

    with tile.TileContext(nc) as tc:
        with (
            tc.tile_pool(name="persist", bufs=1) as pp,
            tc.tile_pool(name="fst", bufs=2) as fpool,
            tc.tile_pool(name="gst", bufs=2) as gpool,
            tc.tile_pool(name="sq", bufs=2) as sqpool,
            tc.tile_pool(name="msg", bufs=4) as mpool,
            tc.tile_pool(name="bitpool", bufs=8) as bpool,
            tc.tile_pool(name="stair", bufs=8) as spool,
            tc.tile_pool(name="outp", bufs=3) as opool,
            tc.tile_pool(name="mpsum", bufs=6, space="PSUM") as mpsum,
            tc.tile_pool(name="hpsum", bufs=2, space="PSUM") as hpsum,
        ):
            # ---------- scalars ----------
            beta_s = pp.tile([1, 1], f32, tag="beta_s")
            eps_s = pp.tile([1, 1], f32, tag="eps_s")
            nc.sync.dma_start(out=beta_s[:], in_=beta_ext[:])
            nc.sync.dma_start(out=eps_s[:], in_=eps_ext[:])
            beta_b = pp.tile([128, 1], f32, tag="beta_b")
            ep1_b = pp.tile([128, 1], f32, tag="ep1_b")
            nc.gpsimd.partition_broadcast(beta_b[:], beta_s[:])
            nc.gpsimd.partition_broadcast(ep1_b[:], eps_s[:])
            nc.vector.tensor_scalar_add(ep1_b[:], ep1_b[:], 1.0)

            # ---------- z ----------
            ztile = pp.tile([128, NT, K], f32, tag="ztile")
            nc.sync.dma_start(
                out=ztile[:].rearrange("p a b -> p (a b)"), in_=zpad_ext[:]
            )
            nc.scalar.activation(
                ztile[:].rearrange("p a b -> p (a b)"),
                ztile[:].rearrange("p a b -> p (a b)"),
                AF.Exp,
                scale=beta_b[:],
            )
            zvec = pp.tile([128, NT], f32, tag="zvec")
            nc.vector.tensor_reduce(zvec[:], ztile[:], mybir.AxisListType.X, ALU.add)
            zrec = pp.tile([128, NT], f32, tag="zrec")
            nc.vector.tensor_scalar_max(zvec[:], zvec[:], 1e-30)
            nc.vector.reciprocal(zrec[:], zvec[:])

            # ---------- g table (bf16), half 0 (partitions 0:64) first ----
            svec = pp.tile([128, NT], f32, tag="svec")
            st_widths = []
            t0 = 0
            while t0 < NT:
                w = min(16, NT - t0)
                st_widths.append((t0, w))
                t0 += w
            for p0, p1 in ((0, 64), (64, 128)):
                for t0, w in st_widths:
                    ft = fpool.tile([128, 16, D], f32, tag="ft")
                    nc.sync.dma_start(
                        out=ft[p0:p1, :w, :],
                        in_=feat_ext[p0:p1, t0 * D : (t0 + w) * D],
                    )
                    sq = sqpool.tile([128, 16, D], f32, tag="sq")
                    nc.scalar.activation(
                        sq[p0:p1, :w, :].rearrange("p a b -> p (a b)"),
                        ft[p0:p1, :w, :].rearrange("p a b -> p (a b)"),
                        AF.Square,
                    )
                    ss = svec[p0:p1, t0 : t0 + w]
                    nc.vector.tensor_reduce(
                        ss, sq[p0:p1, :w, :], mybir.AxisListType.X, ALU.add
                    )
                    nc.vector.tensor_scalar_max(ss, ss, 1e-30)
                    nc.scalar.activation(ss, ss, AF.Sqrt)
                    nc.vector.reciprocal(ss, ss)
                    nc.vector.tensor_tensor(
                        ss, ss, zrec[p0:p1, t0 : t0 + w], ALU.mult
                    )
                    gt = gpool.tile([128, 16, D], bf16, tag="gt")
                    for j in range(w):
                        if j % 2 == 0:
                            nc.vector.tensor_scalar_mul(
                                gt[p0:p1, j, :],
                                ft[p0:p1, j, :],
                                svec[p0:p1, t0 + j : t0 + j + 1],
                            )
                        else:
                            nc.scalar.activation(
                                gt[p0:p1, j, :],
                                ft[p0:p1, j, :],
                                AF.Copy,
                                scale=svec[p0:p1, t0 + j : t0 + j + 1],
                            )
                    nc.sync.dma_start(
                        out=g_dram[p0:p1, t0 * D : (t0 + w) * D],
                        in_=gt[p0:p1, :w, :],
                    )

            # ---------- per-edge weights + bit planes ----------
            wvec = pp.tile([128, tet], f32, tag="wvec")
            nc.sync.dma_start(out=wvec[:], in_=ewp_ext[:])
            nc.scalar.activation(wvec[:], wvec[:], AF.Exp, scale=beta_b[:])
            neg6w = pp.tile([128, tet], f32, tag="neg6w")
            nc.vector.tensor_scalar_mul(neg6w[:], wvec[:], -6.0)

            nbits = pp.tile([14, 128], bf16, tag="nbits")
            nc.sync.dma_start(out=nbits[:], in_=nbits_ext[:])

            gidx_t = pp.tile([128, tet * 8], i16, tag="gidx_t")
            nc.sync.dma_start(out=gidx_t[:], in_=gidx_ext[:])

            g_rows = g_dram[:].rearrange("p (c f) -> (p c) f", f=D)

            # ---------- edge phase: two sweeps (one per table half) -----
            h0sb = pp.tile([128, HTILES, D], f32, tag="h0sb")
            nmax0 = int(net[:, 0].max())
            nmax1 = int(net[:, 1].max())
            # zero all msg slots once: rows skipped by exact-count gathers
            # must stay finite (0 * 0 = 0 in the S@msg matmul)

            T = 0
            gc = 0
            for hh, nmaxh in ((0, nmax0), (1, nmax1)):
                base = 0 if hh == 0 else HALFP
                for i in range(HTILES):
                    nh = int(net[i, hh])
                    msg = mpool.tile([128, max(nmax0, nmax1), D], bf16, tag="msg")
                    nidx = int(nidx_exact[i, hh])
                    nhq = (nidx + 127) // 128
                    if nidx % 128:
                        # rows the exact-count gather skips must be finite;
                        # zero the last column, the gather overwrites the
                        # valid prefix rows
                        nc.vector.memset(msg[:, nhq - 1, :], 0.0)
                    nc.gpsimd.dma_gather(
                        msg[:, :nhq, :],
                        g_rows[base : base + HALFP, :],
                        gidx_t[:, gc : gc + (nidx + 15) // 16],
                        nidx,
                        nidx,
                        D,
                        queue_num=0,
                    )
                    gc += nh * 8
                    hp = hpsum.tile([128, D], f32, tag="hp")
                    bp = bpool.tile([14, max(nmax0, nmax1), 128], bf16, tag="bp")
                    nc.sync.dma_start(
                        out=bp[:, :nh, :],
                        in_=bitp_ext[:, T * 128 : (T + nh) * 128],
                    )
                    for t in range(nh):
                        mp = mpsum.tile([128, 128], f32, tag="mp")
                        nc.tensor.matmul(
                            mp[:], bp[:, t, :], nbits[:], start=True, stop=True
                        )
                        st = spool.tile([128, 128], bf16, tag="st")
                        nc.scalar.activation(
                            st[:],
                            mp[:],
                            AF.Relu,
                            bias=neg6w[:, T + t : T + t + 1],
                            scale=wvec[:, T + t : T + t + 1],
                        )
                        nc.tensor.matmul(
                            hp[:],
                            st[:],
                            msg[:, t, :],
                            start=(t == 0),
                            stop=(t == nh - 1),
                        )
                    T += nh
                    if hh == 0:
                        nc.vector.tensor_copy(h0sb[:, i, :], hp[:])
                    else:
                        rows = min(128, SH - i * 128)
                        ftm = opool.tile([128, D], f32, tag="ftm")
                        ot = opool.tile([128, D], f32, tag="ot")
                        nc.sync.dma_start(
                            out=ftm[:rows, :],
                            in_=featmy_ext[i * 128 : i * 128 + rows, :],
                        )
                        nc.vector.tensor_scalar_mul(
                            ftm[:rows, :], ftm[:rows, :], ep1_b[:rows, :]
                        )
                        nc.vector.tensor_tensor(
                            ot[:rows, :], ftm[:rows, :], hp[:rows, :], ALU.add
                        )
                        nc.vector.tensor_tensor(
                            ot[:rows, :], ot[:rows, :], h0sb[:rows, i, :], ALU.add
                        )
                        nc.sync.dma_start(
                            out=out_ext[i * 128 : i * 128 + rows, :],
                            in_=ot[:rows, :],
                        )

    nc.finalize()
    return nc


def kernel(feat, edge_weight, src, dst, beta, eps):
    from concourse.bass_utils import run_bass_kernel_spmd

    feat = np.asarray(feat, dtype=np.float32)
    ew = np.asarray(edge_weight, dtype=np.float32)
    beta = np.asarray(beta, dtype=np.float32)
    eps = np.asarray(eps, dtype=np.float32)

    zpad_dev, core_arrays, nb, net, nidx_exact, K = _host_prep(src, dst, ew)

    key = (
        K,
        tuple(int(x) for x in net.reshape(-1)),
        tuple(int(x) for x in nidx_exact.reshape(-1)),
    )
    if key not in _COMPILED:
        _COMPILED[key] = _build(net, nidx_exact, K)
    nc = _COMPILED[key]

    feat_pad = np.zeros((NPAD, D), np.float32)
    feat_pad[:N] = feat
    feat_perm = _perm_table(feat_pad)
    beta2 = beta.reshape(1, 1)
    eps2 = eps.reshape(1, 1)

    in_maps = []
    for c in range(NCORES):
        ca = core_arrays[c]
        in_maps.append(
            {
                "feat": feat_perm,
                "feat_my": np.ascontiguousarray(feat[c * SH : (c + 1) * SH]),
                "zpad": zpad_dev,
                "gidx": ca["gidx"],
                "ewp": ca["ewp"],
                "bitp": ca["bitp"],
                "nbits": nb,
                "beta": beta2,
                "eps": eps2,
            }
        )

    res = run_bass_kernel_spmd(nc, in_maps, core_ids=list(range(NCORES)))
    out = np.concatenate([res.results[c]["out"] for c in range(NCORES)], axis=0)
    return out.astype(np.float32)


# revision 43
# speedup vs baseline: 1.2200x; 1.1878x over previous
"""AGNNConv distributed Bass kernel for 8 TRN2 NeuronCores (v2: matmul reduce).

out = (1+eps)*feat + h,  h[d] = sum_{e: dst_e=d} p_e * norm_feat[src_e]
with p_e = edge-softmax grouped by src.

Algebra (softmax max-subtraction dropped -- identity in exact math):
    z_n = sum_{e: src_e=n} exp(beta*ew_e)
    g_n = feat_n / (||feat_n|| * z_n)            # per-node row scale
    h_d = sum_{e: dst_e=d} exp(beta*ew_e) * g[src_e]
    out = (1+eps)*feat + h

Core c owns dst nodes [c*6250, (c+1)*6250). Inputs replicated via in_maps
(free); every core builds the full g table (bf16) locally, then per dst-tile
(128 dst nodes, edges host-grouped + padded):
  1) dma_gather g rows for the tile's edges (bf16, 2 table halves for int16)
  2) one-hot via TensorE bit-trick: M[e,n] = #matching bits of (dst_e, n)
     over 7 bit-planes (K=14 matmul); S = relu(w*M - 6w) fused on ScalarE
     (per-edge w enters via ACT scale/bias) -> w-weighted one-hot, bf16
  3) h_tile (PSUM f32) += S_t^T @ msg_t accumulated over edge tiles
  4) out_tile = h + (1+eps)*feat_my_tile, DMA straight out
No scatter DMA at all: Q7 descriptor generation only for the gather.
"""

import sys

sys.path.insert(0, "/opt/trn_rl_repo")

import numpy as np

N, E, D = 50000, 640000, 128
NCORES = 8
SH = N // NCORES            # 6250 dst nodes per core
NT = (N + 127) // 128       # 391 node tiles (g table)
HALFP = 64 * NT             # permuted-table half base (rows with p<64)
NPAD = NT * 128             # 50048
HTILES = (SH + 127) // 128  # 49 dst tiles per core

PAD_EW = -80.0              # exp(beta*PAD_EW) == 0 (inside ACT LUT range)


def _host_prep(src, dst, edge_weight):
    """Index/layout prep only (no float math on tensor values)."""
    src = np.asarray(src).astype(np.int64)
    dst = np.asarray(dst).astype(np.int64)
    ew = np.asarray(edge_weight).astype(np.float32)

    # ---- global z padding (per-node src-grouped edge weights, fixed K) ----
    deg = np.bincount(src, minlength=NPAD)
    K = int(deg.max())
    order = np.argsort(src, kind="stable")
    starts = np.zeros(NPAD + 1, np.int64)
    np.cumsum(deg, out=starts[1:])
    slot = np.arange(E, dtype=np.int64) - starts[src[order]]
    zpad = np.full((NPAD, K), PAD_EW, np.float32)
    zpad[src[order], slot] = ew[order]
    zpad_dev = np.ascontiguousarray(
        zpad.reshape(NT, 128, K).transpose(1, 0, 2).reshape(128, NT * K)
    )

    # ---- per-core edge grouping by (dst tile, src half) ----
    owner = dst // SH
    dstl = dst - owner * SH           # 0..SH-1
    dtile = dstl // 128               # dst tile
    dbit = dstl % 128                 # within-tile dst row
    srcp = (src % 128) * NT + src // 128  # permuted g-table row
    halfsel = (src % 128 >= 64).astype(np.int64)  # 0 -> rows < HALFP

    counts = np.zeros((NCORES, HTILES, 2), np.int64)
    np.add.at(counts, (owner, dtile, halfsel), 1)
    net = (counts.max(axis=0) + 127) // 128  # [HTILES, 2] edge tiles
    net = np.maximum(net, 1)
    tet = int(net.sum())

    # segment order: (half, tile)
    netT = net.T.reshape(-1)  # [2*HTILES] in (half, tile) order
    seg_off = np.zeros(HTILES * 2, np.int64)
    seg_off[1:] = np.cumsum(netT)[:-1]

    EPAD = tet * 128

    core_arrays = []
    for c in range(NCORES):
        m = np.nonzero(owner == c)[0]
        key = halfsel[m] * HTILES + dtile[m]
        korder = np.argsort(key, kind="stable")
        me = m[korder]
        keys = key[korder]
        kb = np.r_[0, np.nonzero(np.diff(keys))[0] + 1]
        sf = np.zeros(len(keys), np.int64)
        sf[kb] = kb
        np.maximum.accumulate(sf, out=sf)
        within = np.arange(len(keys)) - sf
        pos = seg_off[keys] * 128 + within

        gidx = np.zeros(EPAD, np.int16)          # pads gather row 0 (valid)
        ewp = np.full(EPAD, PAD_EW, np.float32)  # pads weight -> 0
        bits = np.zeros(EPAD, np.int64)

        gbase = halfsel[me] * HALFP
        gidx[pos] = (srcp[me] - gbase).astype(np.int16)
        ewp[pos] = ew[me]
        bits[pos] = dbit[me]

        g16 = gidx.reshape(tet * 8, 16).T        # wrap-16: [16, tet*8]
        g128 = np.tile(g16, (8, 1))              # replicated [128, tet*8]
        ewd = ewp.reshape(tet, 128).T            # [128, tet]

        bp = np.zeros((14, EPAD), np.float32)
        for b in range(7):
            bb = (bits >> b) & 1
            bp[2 * b + 1] = bb
            bp[2 * b] = 1 - bb
        core_arrays.append(
            dict(
                gidx=np.ascontiguousarray(g128),
                ewp=np.ascontiguousarray(ewd),
                bitp=np.ascontiguousarray(bp),
            )
        )

    import ml_dtypes

    nb = np.zeros((14, 128), np.float32)
    nn = np.arange(128)
    for b in range(7):
        bb = (nn >> b) & 1
        nb[2 * b + 1] = bb
        nb[2 * b] = 1 - bb
    nb = nb.astype(ml_dtypes.bfloat16)
    for ca in core_arrays:
        ca["bitp"] = ca["bitp"].astype(ml_dtypes.bfloat16)

    maxc = counts.max(axis=0)  # [HTILES, 2]
    nidx_exact = np.minimum((maxc + 15) // 16 * 16, net * 128)
    nidx_exact = np.maximum(nidx_exact, 16)

    return zpad_dev, core_arrays, nb, net, nidx_exact, K


def _perm_table(feat_pad):
    return np.ascontiguousarray(
        feat_pad.reshape(NT, 128, D).transpose(1, 0, 2).reshape(128, NT * D)
    )


_COMPILED = {}


def _build(net, nidx_exact, K):
    import concourse.bass as bass
    import concourse.bacc as bacc
    from concourse import mybir, tile

    f32 = mybir.dt.float32
    bf16 = mybir.dt.bfloat16
    i16 = mybir.dt.int16
    AF = mybir.ActivationFunctionType
    ALU = mybir.AluOpType

    tet = int(net.sum())
    nmax = int(net.sum(axis=1).max())

    nc = bacc.Bacc(None, debug=False, num_swdge_queues=1)

    feat_ext = nc.dram_tensor("feat", [128, NT * D], f32, kind="ExternalInput")
    featmy_ext = nc.dram_tensor("feat_my", [SH, D], f32, kind="ExternalInput")
    zpad_ext = nc.dram_tensor("zpad", [128, NT * K], f32, kind="ExternalInput")
    gidx_ext = nc.dram_tensor("gidx", [128, tet * 8], i16, kind="ExternalInput")
    ewp_ext = nc.dram_tensor("ewp", [128, tet], f32, kind="ExternalInput")
    bitp_ext = nc.dram_tensor("bitp", [14, tet * 128], bf16, kind="ExternalInput")
    nbits_ext = nc.dram_tensor("nbits", [14, 128], bf16, kind="ExternalInput")
    beta_ext = nc.dram_tensor("beta", [1, 1], f32, kind="ExternalInput")
    eps_ext = nc.dram_tensor("eps", [1, 1], f32, kind="ExternalInput")
    out_ext = nc.dram_tensor("out", [SH, D], f32, kind="ExternalOutput")

    g_dram = nc.dram_tensor("g_table", [128, NT * D], bf16)

    with tile.TileContext(nc) as tc:
        with (
            tc.tile_pool(name="persist", bufs=1) as pp,
            tc.tile_pool(name="fst", bufs=2) as fpool,
            tc.tile_pool(name="gst", bufs=2) as gpool,
            tc.tile_pool(name="sq", bufs=2) as sqpool,
            tc.tile_pool(name="msg", bufs=4) as mpool,
            tc.tile_pool(name="bitpool", bufs=8) as bpool,
            tc.tile_pool(name="stair", bufs=8) as spool,
            tc.tile_pool(name="outp", bufs=3) as opool,
            tc.tile_pool(name="mpsum", bufs=6, space="PSUM") as mpsum,
            tc.tile_pool(name="hpsum", bufs=2, space="PSUM") as hpsum,
        ):
            # ---------- scalars ----------
            beta_s = pp.tile([1, 1], f32, tag="beta_s")
            eps_s = pp.tile([1, 1], f32, tag="eps_s")
            nc.sync.dma_start(out=beta_s[:], in_=beta_ext[:])
            nc.sync.dma_start(out=eps_s[:], in_=eps_ext[:])
            beta_b = pp.tile([128, 1], f32, tag="beta_b")
            ep1_b = pp.tile([128, 1], f32, tag="ep1_b")
            nc.gpsimd.partition_broadcast(beta_b[:], beta_s[:])
            nc.gpsimd.partition_broadcast(ep1_b[:], eps_s[:])
            nc.vector.tensor_scalar_add(ep1_b[:], ep1_b[:], 1.0)

            # ---------- z ----------
            ztile = pp.tile([128, NT, K], f32, tag="ztile")
            nc.sync.dma_start(
                out=ztile[:].rearrange("p a b -> p (a b)"), in_=zpad_ext[:]
            )
            nc.scalar.activation(
                ztile[:].rearrange("p a b -> p (a b)"),
                ztile[:].rearrange("p a b -> p (a b)"),
                AF.Exp,
                scale=beta_b[:],
            )
            zvec = pp.tile([128, NT], f32, tag="zvec")
            nc.vector.tensor_reduce(zvec[:], ztile[:], mybir.AxisListType.X, ALU.add)
            zrec = pp.tile([128, NT], f32, tag="zrec")
            nc.vector.tensor_scalar_max(zvec[:], zvec[:], 1e-30)
            nc.vector.reciprocal(zrec[:], zvec[:])

            # ---------- g table (bf16), half 0 (partitions 0:64) first ----
            svec = pp.tile([128, NT], f32, tag="svec")
            st_widths = []
            t0 = 0
            while t0 < NT:
                w = min(16, NT - t0)
                st_widths.append((t0, w))
                t0 += w
            for p0, p1 in ((0, 64), (64, 128)):
                for t0, w in st_widths:
                    ft = fpool.tile([128, 16, D], f32, tag="ft")
                    nc.sync.dma_start(
                        out=ft[p0:p1, :w, :],
                        in_=feat_ext[p0:p1, t0 * D : (t0 + w) * D],
                    )
                    sq = sqpool.tile([128, 16, D], f32, tag="sq")
                    nc.scalar.activation(
                        sq[p0:p1, :w, :].rearrange("p a b -> p (a b)"),
                        ft[p0:p1, :w, :].rearrange("p a b -> p (a b)"),
                        AF.Square,
                    )
                    ss = svec[p0:p1, t0 : t0 + w]
                    nc.vector.tensor_reduce(
                        ss, sq[p0:p1, :w, :], mybir.AxisListType.X, ALU.add
                    )
                    nc.vector.tensor_scalar_max(ss, ss, 1e-30)
                    nc.scalar.activation(ss, ss, AF.Sqrt)
                    nc.vector.reciprocal(ss, ss)
                    nc.vector.tensor_tensor(
                        ss, ss, zrec[p0:p1, t0 : t0 + w], ALU.mult
                    )
                    gt = gpool.tile([128, 16, D], bf16, tag="gt")
                    for j in range(w):
                        if j % 2 == 0:
                            nc.vector.tensor_scalar_mul(
                                gt[p0:p1, j, :],
                                ft[p0:p1, j, :],
                                svec[p0:p1, t0 + j : t0 + j + 1],
                            )
                        else:
                            nc.scalar.activation(
                                gt[p0:p1, j, :],
                                ft[p0:p1, j, :],
                                AF.Copy,
                                scale=svec[p0:p1, t0 + j : t0 + j + 1],
                            )
                    nc.sync.dma_start(
                        out=g_dram[p0:p1, t0 * D : (t0 + w) * D],
                        in_=gt[p0:p1, :w, :],
                    )

            # ---------- per-edge weights + bit planes ----------
            wvec = pp.tile([128, tet], f32, tag="wvec")
            nc.sync.dma_start(out=wvec[:], in_=ewp_ext[:])
            nc.scalar.activation(wvec[:], wvec[:], AF.Exp, scale=beta_b[:])
            neg6w = pp.tile([128, tet], f32, tag="neg6w")
            nc.vector.tensor_scalar_mul(neg6w[:], wvec[:], -6.0)

            nbits = pp.tile([14, 128], bf16, tag="nbits")
            nc.sync.dma_start(out=nbits[:], in_=nbits_ext[:])

            gidx_t = pp.tile([128, tet * 8], i16, tag="gidx_t")
            nc.sync.dma_start(out=gidx_t[:], in_=gidx_ext[:])

            g_rows = g_dram[:].rearrange("p (c f) -> (p c) f", f=D)

            # ---------- edge phase: two sweeps (one per table half) -----
            h0sb = pp.tile([128, HTILES, D], f32, tag="h0sb")
            nmax0 = int(net[:, 0].max())
            nmax1 = int(net[:, 1].max())
            # zero all msg slots once: rows skipped by exact-count gathers
            # must stay finite (0 * 0 = 0 in the S@msg matmul)

            T = 0
            gc = 0
            MERGE_CAP = 1280
            for hh, nmaxh in ((0, nmax0), (1, nmax1)):
                base = 0 if hh == 0 else HALFP
                i = 0
                while i < HTILES:
                    nh_a = int(net[i, hh])
                    # try to merge tile i with tile i+1 (same half, adjacent
                    # in the host gidx layout): merged descriptor count =
                    # full padded extent of A + exact count of B
                    merged = (
                        i + 1 < HTILES
                        and nh_a * 128 + int(nidx_exact[i + 1, hh]) <= MERGE_CAP
                    )
                    if merged:
                        tiles = [i, i + 1]
                        nidx = nh_a * 128 + int(nidx_exact[i + 1, hh])
                        ncols_total = nh_a + int(net[i + 1, hh])
                    else:
                        tiles = [i]
                        nidx = int(nidx_exact[i, hh])
                        ncols_total = nh_a
                    nhq = (nidx + 127) // 128
                    msg = mpool.tile(
                        [128, 2 * max(nmax0, nmax1), D], bf16, tag="msg"
                    )
                    if nidx % 128:
                        nc.vector.memset(msg[:, nhq - 1, :], 0.0)
                    nc.gpsimd.dma_gather(
                        msg[:, :nhq, :],
                        g_rows[base : base + HALFP, :],
                        gidx_t[:, gc : gc + (nidx + 15) // 16],
                        nidx,
                        nidx,
                        D,
                        queue_num=0,
                    )
                    gc += ncols_total * 8
                    coff = 0
                    for ii in tiles:
                        nh = int(net[ii, hh])
                        hp = hpsum.tile([128, D], f32, tag="hp")
                        bp = bpool.tile(
                            [14, max(nmax0, nmax1), 128], bf16, tag="bp"
                        )
                        nc.sync.dma_start(
                            out=bp[:, :nh, :],
                            in_=bitp_ext[:, T * 128 : (T + nh) * 128],
                        )
                        for t in range(nh):
                            mp = mpsum.tile([128, 128], f32, tag="mp")
                            nc.tensor.matmul(
                                mp[:], bp[:, t, :], nbits[:], start=True, stop=True
                            )
                            st = spool.tile([128, 128], bf16, tag="st")
                            nc.scalar.activation(
                                st[:],
                                mp[:],
                                AF.Relu,
                                bias=neg6w[:, T + t : T + t + 1],
                                scale=wvec[:, T + t : T + t + 1],
                            )
                            nc.tensor.matmul(
                                hp[:],
                                st[:],
                                msg[:, coff + t, :],
                                start=(t == 0),
                                stop=(t == nh - 1),
                            )
                        T += nh
                        coff += nh
                        if hh == 0:
                            nc.vector.tensor_copy(h0sb[:, ii, :], hp[:])
                        else:
                            rows = min(128, SH - ii * 128)
                            ftm = opool.tile([128, D], f32, tag="ftm")
                            ot = opool.tile([128, D], f32, tag="ot")
                            nc.sync.dma_start(
                                out=ftm[:rows, :],
                                in_=featmy_ext[ii * 128 : ii * 128 + rows, :],
                            )
                            nc.vector.tensor_scalar_mul(
                                ftm[:rows, :], ftm[:rows, :], ep1_b[:rows, :]
                            )
                            nc.vector.tensor_tensor(
                                ot[:rows, :], ftm[:rows, :], hp[:rows, :], ALU.add
                            )
                            nc.vector.tensor_tensor(
                                ot[:rows, :],
                                ot[:rows, :],
                                h0sb[:rows, ii, :],
                                ALU.add,
                            )
                            nc.sync.dma_start(
                                out=out_ext[ii * 128 : ii * 128 + rows, :],
                                in_=ot[:rows, :],
                            )
                    i += len(tiles)

    nc.finalize()
    return nc


def kernel(feat, edge_weight, src, dst, beta, eps):
    from concourse.bass_utils import run_bass_kernel_spmd

    feat = np.asarray(feat, dtype=np.float32)
    ew = np.asarray(edge_weight, dtype=np.float32)
    beta = np.asarray(beta, dtype=np.float32)
    eps = np.asarray(eps, dtype=np.float32)

    zpad_dev, core_arrays, nb, net, nidx_exact, K = _host_prep(src, dst, ew)

    key = (
        K,
        tuple(int(x) for x in net.reshape(-1)),
        tuple(int(x) for x in nidx_exact.reshape(-1)),
    )
    if key not in _COMPILED:
        _COMPILED[key] = _build(net, nidx_exact, K)
    nc = _COMPILED[key]

    feat_pad = np.zeros((NPAD, D), np.float32)
    feat_pad[:N] = feat
    feat_perm = _perm_table(feat_pad)
    beta2 = beta.reshape(1, 1)
    eps2 = eps.reshape(1, 1)

    in_maps = []
    for c in range(NCORES):
        ca = core_arrays[c]
        in_maps.append(
            {
                "feat": feat_perm,
                "feat_my": np.ascontiguousarray(feat[c * SH : (c + 1) * SH]),
                "zpad": zpad_dev,
                "gidx": ca["gidx"],
                "ewp": ca["ewp"],
                "bitp": ca["bitp"],
                "nbits": nb,
                "beta": beta2,
                "eps": eps2,
            }
        )

    res = run_bass_kernel_spmd(nc, in_maps, core_ids=list(range(NCORES)))
    out = np.concatenate([res.results[c]["out"] for c in range(NCORES)], axis=0)
    return out.astype(np.float32)
